# revision 26
# baseline (speedup 1.0000x reference)
"""BlockSparseLinear forward on 8 Trainium2 NeuronCores.

Computes out = x @ (weight * expand(block_mask))^T + bias for
x [8192, 4096] f32, weight [4096, 4096] f32, bias [4096] f32,
block_mask [128, 128] int32 (32x32 blocks).

Strategy (data-parallel over rows of x):
  - Each of the 8 cores gets a 1024-row slice of x and the full weight /
    bias / block_mask (replicated).  No collectives needed.
  - On-device per core:
      * x slice is transposed via PE (tensor-engine transpose) into a
        resident SBUF tensor xT [i, n] (rounded to f32r).
      * block_mask is expanded on device into a partition-replicated
        [128, IB/4, OB] helper (mrep) using a small selection matmul.
      * weight rows for each 128-output tile are DMA'd naturally,
        PE-transposed to [i, o] tiles, and masked+rounded to f32r in the
        mandatory PSUM->SBUF eviction (DVE multiply with broadcast AP).
      * f32r matmuls (full PE rate, TF32-grade mantissa, fp32 PSUM
        accumulation) accumulate out^T [o, n] over all i.
      * bias is added during the PSUM->SBUF eviction on the scalar
        engine; out^T is stored contiguously to DRAM.
  - Host reassembles: out[rows_c, :] = outT_c.T.
"""
import os
import sys

import ml_dtypes
import numpy as np

sys.path.insert(0, "/opt/trn_rl_repo")

from contextlib import ExitStack

import concourse.bass as bass
import concourse.mybir as mybir
import concourse.tile as tile
from concourse import bacc
from concourse.bass_utils import run_bass_kernel_spmd

N_CORES = 8
BS = 32

# Filled by kernel() after a profiled run (test harness convenience).
LAST_EXEC_TIME_NS = None
LAST_RESULTS = None

F32 = mybir.dt.float32
BF16 = mybir.dt.bfloat16
F32R = mybir.dt.float32r
I32 = mybir.dt.int32


def _build_program(n_rows, IN, OUT):
    """One SPMD program: per-core inputs x [n_rows, IN], w [OUT, IN],
    bias_r [128, OUT//128], mask [OUT//BS, IN//BS]; output outT [OUT, n_rows]."""
    P = 128
    IT = IN // P          # i tiles (contraction)
    OT = OUT // P         # o tiles
    TG = IT // 4          # i tile groups of 4
    NFREE = min(512, n_rows)
    NG = n_rows // NFREE  # n groups (matmul free dim)
    NT = n_rows // P      # n tiles for transpose phase
    IB = IN // BS         # i blocks
    OB = OUT // BS        # o blocks
    assert IB <= 128 and OB <= 128

    nc = bacc.Bacc("TRN2", target_bir_lowering=False, debug=False,
                   num_devices=N_CORES)
    # x/w declared float32r: DMA rounds to the PE's f32r format in flight,
    # letting transposes and matmuls run in f32r (1 cycle/row at N>=256).
    x_d = nc.dram_tensor("x", [n_rows, IN], F32R, kind="ExternalInput")
    w_d = nc.dram_tensor("w", [OUT, IN], F32R, kind="ExternalInput")
    bias_d = nc.dram_tensor("bias_r", [P, OT], F32, kind="ExternalInput")
    mask_d = nc.dram_tensor("mask", [OB, IB], I32, kind="ExternalInput")
    out_d = nc.dram_tensor("outT", [OUT, n_rows], F32, kind="ExternalOutput")

    ident_d = nc.inline_tensor(np.eye(P, dtype=np.float32), name="ident")

    with tile.TileContext(nc) as tc, ExitStack() as ctx:
        const = ctx.enter_context(tc.tile_pool(name="const", bufs=1))
        xtp = ctx.enter_context(tc.tile_pool(name="xt", bufs=1))
        mrp = ctx.enter_context(tc.tile_pool(name="mrep", bufs=1))
        nat = ctx.enter_context(tc.tile_pool(name="nat", bufs=6))
        wtm = ctx.enter_context(tc.tile_pool(name="wtm", bufs=3))
        osb = ctx.enter_context(tc.tile_pool(name="osb", bufs=3))
        dscr = ctx.enter_context(tc.tile_pool(name="dscr", bufs=1, space="DRAM"))
        ppt = ctx.enter_context(tc.tile_pool(name="ppt", bufs=2, space="PSUM"))
        ppo = ctx.enter_context(tc.tile_pool(name="ppo", bufs=4, space="PSUM"))

        ident = const.tile([P, P], F32R)
        nc.sync.dma_start(ident[:], ident_d[:].bitcast(F32R))
        bias_sb = const.tile([P, OT], F32)
        nc.sync.dma_start(bias_sb[:], bias_d[:])

        HI = IN // 2 if IN > 2048 else IN  # natural tiles split in halves

        def load_nat(src_rows, name):
            """Load [128, IN] natural rows as [128, HI] chunk tiles."""
            halves = []
            for h in range(IN // HI):
                t = nat.tile([P, HI], F32R, tag="nat", name=f"{name}_{h}")
                nc.sync.dma_start(t[:], src_rows[:, h * HI:(h + 1) * HI])
                halves.append(t)
            return halves

        def nat_slice(halves, it):
            """[128, 128] column slice for i-tile `it` of a load_nat set."""
            h, loc = (it * P) // HI, (it * P) % HI
            return halves[h][:, loc:loc + P]

        # Prefetch the first weight tile rows before the x-transpose phase
        # so the main loop starts without waiting behind all x loads.
        w_pre = load_nat(w_d[0:P, :], "wpre")

        # ---- mask expansion: mrep[p, t, ob] = mask[ob, 4t + p//32] ----
        mi = const.tile([OB, IB], I32)
        nc.sync.dma_start(mi[:], mask_d[:])
        mf = const.tile([OB, IB], F32R)
        nc.vector.tensor_copy(mf[:], mi[:])
        mtp = ppt.tile([P, 4, P], F32R, tag="ppt")
        nc.tensor.matmul(mtp[:IB, 0, :OB], mf[:], ident[:OB, :OB],
                         is_transpose=True, start=True, stop=True)
        mt = const.tile([IB, OB], F32)
        nc.vector.tensor_copy(mt[:], mtp[:IB, 0, :OB])
        mt_dram = dscr.tile([IB, OB], F32)
        nc.sync.dma_start(mt_dram[:], mt[:])
        # partition-replicate: mask row ib feeds partitions
        # [32*(ib%4) .. 32*(ib%4)+32) of t-slot ib//4 -- 4 broadcast DMAs
        mrep = mrp.tile([P, IB // 4, OB], F32)
        mt_r = mt_dram[:].rearrange("(t h) o -> h t o", h=4)
        for h in range(4):
            nc.sync.dma_start(
                mrep[h * 32:(h + 1) * 32, :, :],
                mt_r[h].partition_broadcast(32))

        # ---- xT build: xt[p, it, n] = x[n, it*128 + p] (f32r) ----
        xt = xtp.tile([P, IT, n_rows], F32R)
        for nt in range(NT):
            xh = load_nat(x_d[nt * P:(nt + 1) * P, :], "xn")
            for ig in range(IT // 4):
                pxt = ppt.tile([P, 4, P], F32R, tag="ppt")
                for j in range(4):
                    nc.tensor.matmul(pxt[:, j, :], nat_slice(xh, ig * 4 + j),
                                     ident[:], is_transpose=True,
                                     start=(j == 0), stop=(j == 3))
                nc.vector.tensor_copy(
                    xt[:, ig * 4:ig * 4 + 4, nt * P:(nt + 1) * P], pxt[:])

        # ---- main: per o-tile, build masked w^T tiles and accumulate ----
        for ot in range(OT):
            wh = w_pre if ot == 0 else \
                load_nat(w_d[ot * P:(ot + 1) * P, :], "wn")
            po = [ppo.tile([P, NFREE], F32, tag="ppo", name=f"po_{ot}_{ng}")
                  for ng in range(NG)]
            wt_tiles = []
            for tg in range(TG):
                pwt = ppt.tile([P, 4, P], F32R, tag="ppt")
                for j in range(4):
                    nc.tensor.matmul(pwt[:, j, :], nat_slice(wh, tg * 4 + j),
                                     ident[:], is_transpose=True,
                                     start=(j == 0), stop=(j == 3))
                wm = wtm.tile([P, 4, P], F32R, tag="wtm")
                m_ap = mrep[:, tg * 4:tg * 4 + 4, ot * 4:ot * 4 + 4] \
                    .broadcast_to([P, 4, 4, BS])
                nc.vector.tensor_tensor(
                    wm[:].rearrange("p a (b c) -> p a b c", c=BS),
                    pwt[:].rearrange("p a (b c) -> p a b c", c=BS),
                    m_ap, op=mybir.AluOpType.mult)
                wt_tiles.append(wm)
            for tg in range(TG):
                wm = wt_tiles[tg]
                for j in range(4):
                    it = tg * 4 + j
                    for ng in range(NG):
                        nc.tensor.matmul(
                            po[ng][:], wm[:, j, :],
                            xt[:, it, ng * NFREE:(ng + 1) * NFREE],
                            start=(tg == 0 and j == 0),
                            stop=(tg == TG - 1 and j == 3))
            for ng in range(NG):
                ob_t = osb.tile([P, NFREE], F32, tag="osb")
                nc.scalar.activation(ob_t[:], po[ng][:],
                                     mybir.ActivationFunctionType.Identity,
                                     bias=bias_sb[:, ot:ot + 1], scale=1.0)
                nc.sync.dma_start(
                    out_d[ot * P:(ot + 1) * P, ng * NFREE:(ng + 1) * NFREE],
                    ob_t[:])

    nc.finalize()
    return nc


def _build_program_t(n_rows, IN, OUT):
    """Tiled-layout SPMD program.  Per-core inputs:
      xq   [NQ, NG, 128, QI, NFREE]  xq[c,ng,p,it,n] = x[ng*NFREE+n, (c*QI+it)*128+p]
      wq   [OT, 128, IT, 128]        wq[ot,p,it,o]   = weight[ot*128+o, it*128+p]
      bias_r [128, OT], mask [OB, IB] (raw block_mask)
    Output outT [OUT, n_rows] (outT[o, n] = out[n, o]).

    The host supplies x and weight in transposed/tiled layouts (pure
    index permutations); all reference arithmetic -- mask expansion and
    application, matmuls, bias -- runs on device.  Every DMA is
    partition-contiguous (16KB runs).
    """
    P = 128
    IT = IN // P
    OT = OUT // P
    TG = IT // 4
    NFREE = min(512, n_rows)
    NG = n_rows // NFREE
    IB = IN // BS
    OB = OUT // BS
    QI = max(IT // 4, 1)  # i-tiles per x quarter
    NQ = IT // QI
    assert IB <= 128 and OB <= 128

    nc = bacc.Bacc("TRN2", target_bir_lowering=False, debug=False,
                   num_devices=N_CORES)
    xq_d = nc.dram_tensor("xq", [NQ, NG, P, QI, NFREE], F32R,
                          kind="ExternalInput")
    wq_d = nc.dram_tensor("wq", [OT, P, IT, P], F32R, kind="ExternalInput")
    bias_d = nc.dram_tensor("bias_r", [P, OT], F32, kind="ExternalInput")
    mask_d = nc.dram_tensor("mask", [OB, IB], I32, kind="ExternalInput")
    out_d = nc.dram_tensor("outT", [OUT, n_rows], F32, kind="ExternalOutput")

    ident_d = nc.inline_tensor(np.eye(P, dtype=np.float32), name="ident")
    # selection matrices: S[b, k, p] = 1 iff k == 4b + p//32 (bf16-exact)
    KH = min(64, IB)
    s_np = np.zeros((IB, KH // 4, P), dtype=ml_dtypes.bfloat16)
    for b in range(KH // 4):
        for p in range(P):
            k = 4 * b + p // 32
            if k < KH:
                for H in range(IB // KH):
                    s_np[KH * H + k, b, p] = 1.0
    s_d = nc.inline_tensor(s_np, name="smat")

    with tile.TileContext(nc) as tc, ExitStack() as ctx:
        const = ctx.enter_context(tc.tile_pool(name="const", bufs=1))
        xtp = ctx.enter_context(tc.tile_pool(name="xt", bufs=1))
        mrp = ctx.enter_context(tc.tile_pool(name="mrep", bufs=1))
        wnt = ctx.enter_context(tc.tile_pool(name="wnt", bufs=3))
        wtm = ctx.enter_context(tc.tile_pool(name="wtm", bufs=3))
        osb = ctx.enter_context(tc.tile_pool(name="osb", bufs=3))
        ppt = ctx.enter_context(tc.tile_pool(name="ppt", bufs=2, space="PSUM"))
        ppo = ctx.enter_context(tc.tile_pool(name="ppo", bufs=4, space="PSUM"))

        xq = [[xtp.tile([P, QI, NFREE], F32R, name=f"xq_{c}_{ng}",
                        tag=f"xq_{c}_{ng}") for ng in range(NG)]
              for c in range(NQ)]

        def load_xq(c, ng, eng):
            eng.dma_start(xq[c][ng][:], xq_d[c, ng])

        def xq_slice(it, ng):
            return xq[it // QI][ng][:, it % QI, :]

        # mask + consts on the sync ring first (the mask expansion chain
        # gates the first masked-weight multiply)
        mi = const.tile([OB, IB], I32)
        nc.sync.dma_start(mi[:], mask_d[:])
        ident = const.tile([P, P], F32R)
        nc.sync.dma_start(ident[:], ident_d[:].bitcast(F32R))
        bias_sb = const.tile([P, OT], F32)
        nc.sync.dma_start(bias_sb[:], bias_d[:])

        # ACT-ring x quarters start immediately
        for c in range(0, NQ, 2):
            for ng in range(NG):
                load_xq(c, ng, nc.scalar)

        def load_wt(ot, name):
            t = wnt.tile([P, IT, P], F32R, tag="wnt", name=name)
            nc.sync.dma_start(t[:], wq_d[ot])
            return t

        w_pre = {0: load_wt(0, "wpre0")}

        # ---- mask expansion: mrep[p, t, ob] = mask[ob, 4t + p//32] ----
        # maskT via PE transpose, then partition-replication via small
        # selection matmuls (mrep[:, t, :] = S[b].T @ maskT[64H:64H+64]).
        s_sb = wnt.tile([IB, KH // 4, P], BF16, tag="wnt", name="s_sb")
        nc.sync.dma_start(s_sb[:], s_d[:])
        mf = const.tile([OB, IB], F32R)
        nc.vector.tensor_copy(mf[:], mi[:])
        mtp = ppt.tile([P, 4, P], F32R, tag="ppt")
        nc.tensor.matmul(mtp[:IB, 0, :OB], mf[:], ident[:OB, :OB],
                         is_transpose=True, start=True, stop=True)
        mt = const.tile([IB, OB], BF16)
        nc.vector.tensor_copy(mt[:], mtp[:IB, 0, :OB])
        mrep = mrp.tile([P, IB // 4, OB], F32)
        for t in range(IB // 4):
            H, b = (t * 4) // KH, (t * 4) % KH // 4
            mps = ppt.tile([P, 4, P], F32, tag="pptm", name=f"mps_{t}")
            nc.tensor.matmul(mps[:, 0, :OB],
                             s_sb[KH * H:KH * (H + 1), b, :],
                             mt[KH * H:KH * (H + 1), :],
                             start=True, stop=True)
            nc.vector.tensor_copy(mrep[:, t, :], mps[:, 0, :OB])

        # sync-ring x quarters + deeper weight prefetch
        if NQ > 1:
            for ng in range(NG):
                load_xq(1, ng, nc.sync)
        if OT > 1:
            w_pre[1] = load_wt(1, "wpre1")
        for c in range(3, NQ, 2):
            for ng in range(NG):
                load_xq(c, ng, nc.sync)
        if OT > 2:
            w_pre[2] = load_wt(2, "wpre2")

        # ---- main loop ----
        for ot in range(OT):
            wn = w_pre[ot] if ot in w_pre else load_wt(ot, "wn")
            po = [ppo.tile([P, NFREE], F32, tag="ppo", name=f"po_{ot}_{ng}")
                  for ng in range(NG)]
            wt_tiles = []
            for tg in range(TG):
                wm = wtm.tile([P, 4, P], F32R, tag="wtm")
                m_ap = mrep[:, tg * 4:tg * 4 + 4, ot * 4:ot * 4 + 4] \
                    .broadcast_to([P, 4, 4, BS])
                nc.vector.tensor_tensor(
                    wm[:].rearrange("p a (b c) -> p a b c", c=BS),
                    wn[:, tg * 4:tg * 4 + 4, :]
                    .rearrange("p a (b c) -> p a b c", c=BS),
                    m_ap, op=mybir.AluOpType.mult)
                wt_tiles.append(wm)
            for tg in range(TG):
                wm = wt_tiles[tg]
                for j in range(4):
                    it = tg * 4 + j
                    for ng in range(NG):
                        nc.tensor.matmul(
                            po[ng][:], wm[:, j, :], xq_slice(it, ng),
                            start=(tg == 0 and j == 0),
                            stop=(tg == TG - 1 and j == 3))
            for ng in range(NG):
                ob_t = osb.tile([P, NFREE], F32, tag="osb")
                nc.scalar.activation(ob_t[:], po[ng][:],
                                     mybir.ActivationFunctionType.Identity,
                                     bias=bias_sb[:, ot:ot + 1], scale=1.0)
                nc.sync.dma_start(
                    out_d[ot * P:(ot + 1) * P, ng * NFREE:(ng + 1) * NFREE],
                    ob_t[:])

    nc.finalize()
    return nc


def _tile_inputs(x_slice, IN, OUT, n_rows):
    """Host layout prep (pure index permutation) for one core's x slice."""
    P = 128
    IT = IN // P
    QI = max(IT // 4, 1)
    NQ = IT // QI
    NFREE = min(512, n_rows)
    NG = n_rows // NFREE
    # xq[c, ng, p, it, n] = x[ng*NFREE+n, (c*QI+it)*P+p]
    xt = x_slice.T                                    # [IN, n_rows]
    xq = xt.reshape(NQ, QI, P, NG, NFREE).transpose(0, 3, 2, 1, 4)
    return np.ascontiguousarray(xq)


def _install_profile_hook():
    """Provide antenv.axon_hooks + the ctypes NTFF hook (profiling only).

    The agent image's antenv package lacks axon_hooks, so trace=True in
    run_bass_kernel_spmd would crash on import.  Recreate the tiny
    get/set module and install the hook trn_boot would have installed.
    """
    import types

    try:
        from antenv import axon_hooks  # noqa: F401
    except ImportError:
        import antenv

        mod = types.ModuleType("antenv.axon_hooks")
        _h = [None]
        mod.set_axon_ntff_profile_hook = lambda h: _h.__setitem__(0, h)
        mod.get_axon_ntff_profile_hook = lambda: _h[0]
        sys.modules["antenv.axon_hooks"] = mod
        antenv.axon_hooks = mod
    from antenv.axon_hooks import (
        get_axon_ntff_profile_hook,
        set_axon_ntff_profile_hook,
    )

    if get_axon_ntff_profile_hook() is None:
        so_path = "/opt/axon/libaxon_pjrt.so"
        if os.path.exists(so_path):
            from trn_agent_boot.trn_boot import _ntff_profile_via_ctypes

            set_axon_ntff_profile_hook(_ntff_profile_via_ctypes(so_path))

    # Zero-egress container: artifact upload would fail; keep it local.
    import concourse.bass_utils as bu

    bu.upload_artifacts = lambda tmpdir: tmpdir


def kernel(x, weight, bias, block_mask):
    global LAST_EXEC_TIME_NS, LAST_RESULTS
    x = np.ascontiguousarray(np.asarray(x, dtype=np.float32))
    weight = np.ascontiguousarray(np.asarray(weight, dtype=np.float32))
    bias = np.asarray(bias, dtype=np.float32)
    block_mask = np.ascontiguousarray(np.asarray(block_mask, dtype=np.int32))

    N, IN = x.shape
    OUT = weight.shape[0]
    assert N % N_CORES == 0
    n_rows = N // N_CORES

    bias_r = np.ascontiguousarray(bias.reshape(OUT // 128, 128).T)
    device_transpose = bool(int(os.environ.get("BSL_DEVICE_TRANSPOSE", "0")))
    if device_transpose:
        nc = _build_program(n_rows, IN, OUT)
        in_maps = [{
            "x": x[c * n_rows:(c + 1) * n_rows, :],
            "w": weight,
            "bias_r": bias_r,
            "mask": block_mask,
        } for c in range(N_CORES)]
    else:
        P, IT, OT = 128, IN // 128, OUT // 128
        # wq[ot, p, it, o] = weight[ot*128+o, it*128+p]
        wq = np.ascontiguousarray(
            weight.reshape(OT, P, IT, P).transpose(0, 3, 2, 1))
        nc = _build_program_t(n_rows, IN, OUT)
        in_maps = [{
            "xq": _tile_inputs(x[c * n_rows:(c + 1) * n_rows, :], IN, OUT,
                               n_rows),
            "wq": wq,
            "bias_r": bias_r,
            "mask": block_mask,
        } for c in range(N_CORES)]

    trace = bool(int(os.environ.get("BASS_KERNEL_TRACE", "0")))
    if trace:
        _install_profile_hook()
    res = run_bass_kernel_spmd(nc, in_maps, list(range(N_CORES)), trace=trace)
    LAST_EXEC_TIME_NS = res.exec_time_ns
    LAST_RESULTS = res

    out = np.empty((N, OUT), dtype=np.float32)
    for c in range(N_CORES):
        out[c * n_rows:(c + 1) * n_rows, :] = res.results[c]["outT"].T
    return out
